# revision 40
# baseline (speedup 1.0000x reference)
"""Mistral GQA self-attention on 8 NeuronCores, tensor-parallel over heads.

Sharding: core c owns q-heads [4c, 4c+4) and kv-head c (q_group-aligned).
Each core computes its heads' attention output and a partial output
projection (rows 512c..512c+512 of wo); host sums the 8 partials.

v2 device scheme (per core):
  - QKV projections and the output projection run in error-compensated
    fp8 (e4m3) with DoubleRow perf mode: each operand is split into
    fp8 hi + fp8 lo (host-side for x/w/wo; on-device DVE split for the
    attention output), and 3 of the 4 cross terms are computed.
    DoubleRow packs 2 contraction k-tiles per instruction at 0.5
    cycles/row -> 4x fp16 throughput per k-tile; 3 terms -> 1.33x net,
    with ~0.2% relative error per stage.
  - Scale folding: wq premult by ATTN_SCALE*512, wk/wv by 512 (fp8
    range); cos/sin tables divided by 512 (un-scales q,k in RoPE); the
    all-ones Z matrix holds 512 so 1/Z cancels v's 512; wo premult by
    64, undone on the host after the partial sum.
  - V is projected directly in [t, d] layout (lhsT=x chunk, rhs=wv
    chunk) - no transposes.
  - Projection outputs (k, v, q0..q3) run as sequential 48-matmul
    chains through a single 2-bank PSUM tile; x tiles stay resident in
    SBUF per token-group so each output re-reads them.
  - Attention: S^T[k,q] blocks fp16 (kt.T @ qt), strictly-upper blocks
    skipped, diagonal blocks get a single const 128x128 lower-triangle
    -30000 tile added on the [q0,q0+128) band before exp (exp->0 there).
    Softmax without max-subtraction; exp(S-4) keeps e in fp16 range.
    Z row-sums via ones-matmuls (value 512), O^T = sum_k V_blk @ e.
  - DMA layout/queues: x and weights are host-packed as SBUF partition
    images (hi|lo fp8 adjacent per row) so chunks load in few
    128-descriptor DMAs; x streams + out writes issue from SP, RoPE
    rotate copies from Act - bounds head-of-line blocking; a small
    double-buffered xpre tile prefetches the first 12 contraction
    chunks of the next token-group.
  - Output written fp16 (values 64x true); host sums 8 partials /64.
"""
import sys

sys.path.insert(0, "/opt/trn_rl_repo")
import numpy as np

B, T, H, D = 2, 2048, 32, 128
Q_GROUP = 4
H_KV = H // Q_GROUP
INNER = H * D          # 4096
NCORES = 8
HPC = H // NCORES      # 4 q-heads per core
ATTN_SCALE = 1.0 / np.sqrt(D)
BT = B * T             # 4096
QG = 512               # q-group (free dim of attention matmuls)
NQG = T // QG          # 4
NKB = T // 128         # 16 k-blocks
NIC = INNER // 128     # 32 contraction chunks
NICP = NIC // 2        # 16 contraction chunk pairs
NTB = T // 128         # 16 token blocks per batch
NCG = INNER // 512     # 8 output column groups
SW = 512.0             # qkv weight prescale (fp8 range + fold)
SO = 64.0              # wo prescale, undone on host

_built = {}
BUILD_MARKS = []


def _split_waits(nc, mybir):
    """Walrus codegen in this container supports only 1 sync-wait per ISA
    instruction; hoist extra waits onto preceding same-engine EventSemaphore
    instructions (1 wait each)."""
    for f in nc.m.functions:
        for bb in f.blocks:
            new = []
            for inst in bb.instructions:
                si = inst.sync_info
                ow = list(si.on_wait) if si is not None and si.on_wait else []
                if len(ow) > 1:
                    for wi, w in enumerate(ow):
                        ev = mybir.InstEventSemaphore(
                            name=f"{inst.name}-wsplit{wi}",
                            ins=[], outs=[],
                            sync_info=mybir.SyncInfo(on_wait=[w], on_update=[]),
                        )
                        ev.engine = inst.engine
                        ev.debug = inst.debug
                        new.append(ev)
                    inst.sync_info = mybir.SyncInfo(
                        on_wait=[], on_update=list(si.on_update or []))
                new.append(inst)
            bb.instructions[:] = new


def _build_causal():
    import concourse.bass as bass
    import concourse.mybir as mybir
    import concourse.tile as tile
    from contextlib import ExitStack

    F32 = mybir.dt.float32
    F16 = mybir.dt.float16
    F8 = mybir.dt.float8e4
    DR = mybir.MatmulPerfMode.DoubleRow
    EXP = mybir.ActivationFunctionType.Exp

    nc = bass.Bass(trn_type="TRN2", target_bir_lowering=False, debug=False)
    xpk = nc.dram_tensor("xpk", [128, B * NQG, NIC, 2 * QG], F8,
                         kind="ExternalInput").ap()
    wh = nc.dram_tensor("wh", [128, NIC, (HPC + 2) * D], F8,
                        kind="ExternalInput").ap()
    wl = nc.dram_tensor("wl", [128, NIC, (HPC + 2) * D], F8,
                        kind="ExternalInput").ap()
    woh = nc.dram_tensor("woh", [128, HPC, INNER], F8,
                         kind="ExternalInput").ap()
    wol = nc.dram_tensor("wol", [128, HPC, INNER], F8,
                         kind="ExternalInput").ap()
    cosT = nc.dram_tensor("cosT", [D, T], F16, kind="ExternalInput").ap()
    sinTs = nc.dram_tensor("sinTs", [D, T], F16, kind="ExternalInput").ap()
    trineg = nc.dram_tensor("trineg", [128, 128], F16,
                            kind="ExternalInput").ap()
    out = nc.dram_tensor("out", [BT, INNER], F16, kind="ExternalOutput").ap()

    KCOL = HPC * D          # k output column offset in w
    VCOL = (HPC + 1) * D    # v output column offset

    def mark(label):
        BUILD_MARKS.append((label, nc.next_id()))

    with tile.TileContext(nc) as tc, ExitStack() as ctx:
        const = ctx.enter_context(tc.tile_pool(name="const", bufs=1))
        w_h = const.tile([128, NIC, (HPC + 2) * D], F8, name="w_h")
        w_l = const.tile([128, NIC, (HPC + 2) * D], F8, name="w_l")
        cos_sb = const.tile([D, T], F16, name="cos")
        sin_sb = const.tile([D, T], F16, name="sin")
        tri_sb = const.tile([128, 128], F16, name="tri")
        ones_col = const.tile([128, 128], F16, name="ones")
        nc.gpsimd.memset(ones_col, SW)
        biasm4 = const.tile([128, 1], F32, name="bias")
        nc.gpsimd.memset(biasm4, -4.0)
        wo_h = const.tile([128, HPC, INNER], F8, name="wo_h")
        wo_l = const.tile([128, HPC, INNER], F8, name="wo_l")

        xpool = ctx.enter_context(tc.tile_pool(name="xres", bufs=1))
        xpre = ctx.enter_context(tc.tile_pool(name="xpre", bufs=2))
        otpool = ctx.enter_context(tc.tile_pool(name="ot8", bufs=1))
        wosb = ctx.enter_context(tc.tile_pool(name="wos", bufs=4))

        for b in range(B):
            t0 = b * T
            with tc.tile_pool(name=f"bp{b}", bufs=1) as bp, \
                 tc.tile_pool(name="exps", bufs=4) as spool, \
                 tc.tile_pool(name="asml", bufs=1) as apool:
                qt_sb = bp.tile([D, HPC, T], F16, name=f"qt{b}")
                kt_sb = bp.tile([D, T], F16, name=f"kt{b}")
                v_sb = bp.tile([128, NKB, D], F16, name=f"v{b}")
                projctx = tc.tile_pool(name="pps", bufs=2, space="PSUM")
                pps = projctx.__enter__()
                epsctx = tc.tile_pool(name="peps", bufs=3)
                epool = epsctx.__enter__()
                # ---------------- projections ----------------
                for tg in range(NQG):
                    mark(f"proj b{b} tg{tg}")
                    prj = pps.tile([128, 2, 512], F32, tag="prj")
                    g = b * NQG + tg
                    xpre_sb = xpre.tile([128, 12, 2 * QG], F8, tag="xp")
                    x_sb = xpool.tile([128, NIC - 12, 2 * QG], F8, tag="x")
                    for q in range(16):
                        if b == 0 and tg == 0 and q % 2 == 0:
                            qq = q // 2
                            nc.sync.dma_start(
                                out=w_h[:, 4 * qq:4 * qq + 4, :],
                                in_=wh[:, 4 * qq:4 * qq + 4, :])
                        if q < 6:
                            nc.sync.dma_start(
                                out=xpre_sb[:, 2 * q:2 * q + 2, :],
                                in_=xpk[:, g, 2 * q:2 * q + 2, :])
                        else:
                            nc.sync.dma_start(
                                out=x_sb[:, 2 * q - 12:2 * q - 10, :],
                                in_=xpk[:, g, 2 * q:2 * q + 2, :])
                        if b == 0 and tg == 0 and q % 2 == 1:
                            qq = q // 2
                            nc.sync.dma_start(
                                out=w_l[:, 4 * qq:4 * qq + 4, :],
                                in_=wl[:, 4 * qq:4 * qq + 4, :])
                    if b == 0 and tg == 0:
                        nc.sync.dma_start(out=cos_sb, in_=cosT)
                        nc.sync.dma_start(out=sin_sb, in_=sinTs)
                        nc.sync.dma_start(out=tri_sb, in_=trineg)

                    def xsl(icp, lo, cs):
                        if icp < 6:
                            base, idx = xpre_sb, 2 * icp
                        else:
                            base, idx = x_sb, 2 * icp - 12
                        o = QG if lo else 0
                        return base[:, idx:idx + 2, o + cs.start:o + cs.stop]

                    terms = [(w_h, False), (w_l, False), (w_h, True)]
                    # output order: k, v, q0..q3 ; psum half rotates
                    half = 0
                    mark(f"kproj b{b} tg{tg}")
                    # k projection
                    for icp in range(NICP):
                        for ti, (wt, lo) in enumerate(terms):
                            nc.tensor.matmul(
                                prj[:, half, :],
                                wt[:, 2 * icp:2 * icp + 2, KCOL:KCOL + D],
                                xsl(icp, lo, slice(0, QG)),
                                start=(ti == 0 and icp == 0),
                                stop=(ti == 2 and icp == NICP - 1),
                                perf_mode=DR)
                    k_ps = prj[:, half, :]
                    half ^= 1
                    # v projection, [t, d] layout, 4 token sub-blocks
                    mark(f"vproj b{b} tg{tg}")
                    for j in range(4):
                        for icp in range(NICP):
                            for ti, (wt, lo) in enumerate(terms):
                                nc.tensor.matmul(
                                    prj[:, half, j * 128:(j + 1) * 128],
                                    xsl(icp, lo,
                                        slice(j * 128, (j + 1) * 128)),
                                    wt[:, 2 * icp:2 * icp + 2,
                                       VCOL:VCOL + D],
                                    start=(ti == 0 and icp == 0),
                                    stop=(ti == 2 and icp == NICP - 1),
                                    perf_mode=DR)
                    for j in range(4):
                        nc.scalar.copy(v_sb[:, 4 * tg + j, :],
                                       prj[:, half, j * 128:(j + 1) * 128])
                    half ^= 1
                    # rope for k (uses k_ps while v chain runs on PE)
                    cs = cos_sb[:, tg * QG:(tg + 1) * QG]
                    ss = sin_sb[:, tg * QG:(tg + 1) * QG]

                    def rope(src_ps, dst):
                        p_sb = epool.tile([128, QG], F16, tag="psb")
                        nc.scalar.copy(p_sb, src_ps)
                        rot = epool.tile([128, QG], F16, tag="rot")
                        nc.scalar.dma_start(out=rot[0:64, :],
                                            in_=p_sb[64:128, :])
                        nc.scalar.dma_start(out=rot[64:128, :],
                                            in_=p_sb[0:64, :])
                        a_t = epool.tile([128, QG], F16, tag="ra")
                        nc.vector.tensor_mul(a_t, p_sb, cs)
                        b_t = epool.tile([128, QG], F16, tag="rb")
                        nc.vector.tensor_mul(b_t, rot, ss)
                        nc.vector.tensor_add(dst, a_t, b_t)

                    rope(k_ps, kt_sb[:, tg * QG:(tg + 1) * QG])
                    mark(f"qproj b{b} tg{tg}")
                    # q projections
                    for i in range(HPC):
                        if i % 2 == 0:
                            prj = pps.tile([128, 2, 512], F32, tag="prj")
                            half = 0
                        c0 = i * D
                        for icp in range(NICP):
                            for ti, (wt, lo) in enumerate(terms):
                                nc.tensor.matmul(
                                    prj[:, half, :],
                                    wt[:, 2 * icp:2 * icp + 2, c0:c0 + D],
                                    xsl(icp, lo, slice(0, QG)),
                                    start=(ti == 0 and icp == 0),
                                    stop=(ti == 2 and icp == NICP - 1),
                                    perf_mode=DR)
                        rope(prj[:, half, :],
                             qt_sb[:, i, tg * QG:(tg + 1) * QG])
                        half ^= 1

                epsctx.__exit__(None, None, None)
                projctx.__exit__(None, None, None)

                # ---------------- attention ----------------
                oth = otpool.tile([D, HPC, T], F8, tag="oth")
                otl = otpool.tile([D, HPC, T], F8, tag="otl")
                with tc.tile_pool(name="sps", bufs=5, space="PSUM") as sps, \
                     tc.tile_pool(name="ops", bufs=2, space="PSUM") as ops, \
                     tc.tile_pool(name="zps", bufs=1, space="PSUM") as zps:
                    for h in range(HPC):
                        if b == 0 and h == 1:
                            nc.sync.dma_start(out=wo_h, in_=woh)
                            nc.sync.dma_start(out=wo_l, in_=wol)
                        for qg in range(NQG):
                            mark(f"attn b{b} h{h} qg{qg}")
                            qs = qt_sb[:, h, qg * QG:(qg + 1) * QG]
                            kmax = 4 * qg + 4
                            o_ps = ops.tile([D, QG], F32, tag="o")
                            z_ps = zps.tile([128, QG], F32, tag="z")
                            es = []

                            def emit_s(kb):
                                q0 = max(0, 128 * (kb - 4 * qg))
                                s_ps = sps.tile([128, QG], F32, tag="s")
                                nc.tensor.matmul(
                                    s_ps[:, q0:],
                                    kt_sb[:, kb * 128:(kb + 1) * 128],
                                    qs[:, q0:], start=True, stop=True)
                                if kb >= 4 * qg:
                                    nc.vector.tensor_add(
                                        s_ps[:, q0:q0 + 128],
                                        s_ps[:, q0:q0 + 128], tri_sb)
                                e_sb = spool.tile([128, QG], F16, tag="e")
                                nc.scalar.activation(
                                    e_sb[:, q0:], s_ps[:, q0:], EXP,
                                    bias=biasm4)
                                es.append((e_sb, q0))

                            def emit_zo(kb):
                                e_sb, q0 = es[kb]
                                nc.tensor.matmul(
                                    z_ps[:, q0:], ones_col, e_sb[:, q0:],
                                    start=(kb == 0), stop=(kb == kmax - 1))
                                nc.tensor.matmul(
                                    o_ps[:, q0:], v_sb[:, kb, :],
                                    e_sb[:, q0:],
                                    start=(kb == 0), stop=(kb == kmax - 1))

                            LA = min(3, kmax - 1)
                            for kb in range(LA):
                                emit_s(kb)
                            for kb in range(LA, kmax):
                                emit_s(kb)
                                emit_zo(kb - LA)
                            for kb in range(kmax - LA, kmax):
                                emit_zo(kb)
                            r_sb = apool.tile([128, QG], F32, tag="r")
                            nc.vector.reciprocal(r_sb, z_ps)
                            o16 = apool.tile([128, QG], F16, tag="o16")
                            nc.vector.tensor_mul(o16, o_ps, r_sb)
                            ohs = oth[:, h, qg * QG:(qg + 1) * QG]
                            nc.vector.tensor_copy(ohs, o16)
                            nc.vector.tensor_sub(
                                otl[:, h, qg * QG:(qg + 1) * QG], o16, ohs)

                # ---------------- output projection ----------------
                wopsctx = tc.tile_pool(name="wops", bufs=3, space="PSUM")
                wops = wopsctx.__enter__()
                for tb in range(NTB):
                    mark(f"wo b{b} tb{tb}")
                    tr = slice(tb * 128, (tb + 1) * 128)
                    for cg in range(NCG):
                        half = cg % 2
                        if half == 0:
                            wop_t = wops.tile([128, 2, 512], F32, tag="wop")
                        cr = slice(cg * 512, (cg + 1) * 512)
                        woterms = [(oth, wo_h), (otl, wo_h), (oth, wo_l)]
                        for ti, (ot_t, wo_t) in enumerate(woterms):
                            for hp in range(2):
                                nc.tensor.matmul(
                                    wop_t[:, half, :],
                                    ot_t[:, 2 * hp:2 * hp + 2, tr],
                                    wo_t[:, 2 * hp:2 * hp + 2, cr],
                                    start=(ti == 0 and hp == 0),
                                    stop=(ti == 2 and hp == 1),
                                    perf_mode=DR)
                        if half == 1:
                            o_sb = wosb.tile([128, 2, 512], F16, tag="osb")
                            if (tb * 4 + cg // 2) % 2 == 0:
                                nc.scalar.copy(o_sb, wop_t)
                            else:
                                nc.vector.tensor_copy(o_sb, wop_t)
                            nc.sync.dma_start(
                                out=out[t0 + tb * 128:t0 + (tb + 1) * 128,
                                        (cg - 1) * 512:(cg + 1) * 512],
                                in_=o_sb)
                wopsctx.__exit__(None, None, None)
    import concourse.mybir as mybir
    _split_waits(nc, mybir)
    return nc


def _build_generic():
    """Fallback for non-canonical masks: fp16 everywhere (original v1)."""
    import concourse.bass as bass
    import concourse.mybir as mybir
    import concourse.tile as tile
    from concourse.masks import make_identity
    from contextlib import ExitStack

    F32 = mybir.dt.float32
    FR = mybir.dt.float16
    EXP = mybir.ActivationFunctionType.Exp

    nc = bass.Bass(trn_type="TRN2", target_bir_lowering=False, debug=False)
    xT = nc.dram_tensor("xT", [INNER, BT], FR, kind="ExternalInput").ap()
    wq = nc.dram_tensor("wq", [INNER, HPC * D], FR, kind="ExternalInput").ap()
    wkv = nc.dram_tensor("wkv", [INNER, 2 * D], FR, kind="ExternalInput").ap()
    wo = nc.dram_tensor("wo", [HPC * D, INNER], FR, kind="ExternalInput").ap()
    cosT = nc.dram_tensor("cosT", [D, T], F32, kind="ExternalInput").ap()
    sinTs = nc.dram_tensor("sinTs", [D, T], F32, kind="ExternalInput").ap()
    mwTf = nc.dram_tensor("mwTf", [T, T], F32, kind="ExternalInput").ap()
    mbTf = nc.dram_tensor("mbTf", [T, T], F32, kind="ExternalInput").ap()
    out = nc.dram_tensor("out", [BT, INNER], F32, kind="ExternalOutput").ap()

    def mark(label):
        BUILD_MARKS.append((label, nc.next_id()))

    with tile.TileContext(nc) as tc, ExitStack() as ctx:
        const = ctx.enter_context(tc.tile_pool(name="const", bufs=1))
        cos_sb = const.tile([D, T], F32)
        sin_sb = const.tile([D, T], F32)
        nc.sync.dma_start(out=cos_sb, in_=cosT)
        nc.sync.dma_start(out=sin_sb, in_=sinTs)
        ones_col = const.tile([128, 128], FR)
        nc.gpsimd.memset(ones_col, 1.0)
        ident = const.tile([128, 128], F32)
        make_identity(nc, ident)
        biasm4 = const.tile([128, 1], F32)
        nc.gpsimd.memset(biasm4, -4.0)

        for b in range(B):
            t0 = b * T
            with tc.tile_pool(name=f"bp{b}", bufs=1) as bp:
                qt_sb = [bp.tile([D, T], FR, tag=f"qt{h}", name=f"qt{h}")
                         for h in range(HPC)]
                kt_sb = bp.tile([D, T], FR, tag="kt")
                v_sb = bp.tile([128, NKB, D], FR, tag="v")
                with tc.tile_pool(name="wproj", bufs=1) as wpool, \
                     tc.tile_pool(name="xin", bufs=8) as xpool, \
                     tc.tile_pool(name="peps", bufs=3) as epool, \
                     tc.tile_pool(name="pps", bufs=1, space="PSUM") as pps, \
                     tc.tile_pool(name="tps", bufs=2, space="PSUM") as tps:
                    w_sb = wpool.tile([128, NIC, (HPC + 2) * D], FR)
                    for ic in range(NIC):
                        nc.sync.dma_start(
                            out=w_sb[:, ic, : HPC * D],
                            in_=wq[ic * 128:(ic + 1) * 128, :])
                        nc.sync.dma_start(
                            out=w_sb[:, ic, HPC * D:],
                            in_=wkv[ic * 128:(ic + 1) * 128, :])
                    for tg in range(NQG):
                        tc0 = t0 + tg * QG
                        prj = [pps.tile([128, QG], F32, tag=f"prj{i}",
                                        name=f"prj{i}")
                               for i in range(HPC + 2)]
                        for ic in range(NIC):
                            x_sb = xpool.tile([128, QG], FR)
                            nc.sync.dma_start(
                                out=x_sb,
                                in_=xT[ic * 128:(ic + 1) * 128, tc0:tc0 + QG])
                            for i in range(HPC + 2):
                                nc.tensor.matmul(
                                    prj[i],
                                    w_sb[:, ic, i * D:(i + 1) * D],
                                    x_sb,
                                    start=(ic == 0), stop=(ic == NIC - 1))
                        cs = cos_sb[:, tg * QG:(tg + 1) * QG]
                        ss = sin_sb[:, tg * QG:(tg + 1) * QG]
                        for i in range(HPC + 1):
                            ps = prj[i]
                            p_sb = epool.tile([128, QG], F32, tag="psb")
                            nc.scalar.copy(p_sb, ps)
                            rot = epool.tile([128, QG], F32, tag="rot")
                            nc.sync.dma_start(out=rot[0:64, :],
                                              in_=p_sb[64:128, :])
                            nc.sync.dma_start(out=rot[64:128, :],
                                              in_=p_sb[0:64, :])
                            a_t = epool.tile([128, QG], F32, tag="ropea")
                            nc.vector.tensor_mul(a_t, p_sb, cs)
                            b_t = epool.tile([128, QG], F32, tag="ropeb")
                            nc.vector.tensor_mul(b_t, rot, ss)
                            dst = qt_sb[i] if i < HPC else kt_sb
                            nc.vector.tensor_add(
                                dst[:, tg * QG:(tg + 1) * QG], a_t, b_t)
                        vtmp = epool.tile([128, QG], F32, tag="vtmp")
                        nc.scalar.copy(vtmp, prj[HPC + 1])
                        for j in range(QG // 128):
                            vt_ps = tps.tile([128, 128], F32, tag="vt")
                            nc.tensor.transpose(
                                vt_ps, vtmp[:, j * 128:(j + 1) * 128], ident)
                            nc.vector.tensor_copy(v_sb[:, tg * 4 + j, :],
                                                  vt_ps)

                ot_sb = [bp.tile([D, T], FR, tag=f"ot{h}", name=f"ot{h}")
                         for h in range(HPC)]
                with tc.tile_pool(name="exps", bufs=24) as spool, \
                     tc.tile_pool(name="asml", bufs=4) as apool, \
                     tc.tile_pool(name="sps", bufs=5, space="PSUM") as sps, \
                     tc.tile_pool(name="ops", bufs=2, space="PSUM") as ops, \
                     tc.tile_pool(name="zps", bufs=1, space="PSUM") as zps:
                    for h in range(HPC):
                        for qg in range(NQG):
                            qs = qt_sb[h][:, qg * QG:(qg + 1) * QG]
                            kmax = NKB
                            o_ps = ops.tile([D, QG], F32, tag="o")
                            z_ps = zps.tile([128, QG], F32, tag="z")
                            for kb in range(kmax):
                                s_ps = sps.tile([128, QG], F32, tag="s")
                                nc.tensor.matmul(
                                    s_ps,
                                    kt_sb[:, kb * 128:(kb + 1) * 128],
                                    qs, start=True, stop=True)
                                mw_t = apool.tile([128, QG], F32, tag="mw")
                                nc.sync.dma_start(
                                    out=mw_t,
                                    in_=mwTf[kb * 128:(kb + 1) * 128,
                                             qg * QG:(qg + 1) * QG])
                                mb_t = apool.tile([128, QG], F32, tag="mb")
                                nc.sync.dma_start(
                                    out=mb_t,
                                    in_=mbTf[kb * 128:(kb + 1) * 128,
                                             qg * QG:(qg + 1) * QG])
                                nc.vector.tensor_mul(s_ps, s_ps, mw_t)
                                nc.vector.tensor_add(s_ps, s_ps, mb_t)
                                e_sb = spool.tile([128, QG], FR, tag="e")
                                nc.scalar.activation(e_sb, s_ps, EXP,
                                                     bias=biasm4)
                                nc.tensor.matmul(
                                    z_ps, ones_col, e_sb,
                                    start=(kb == 0), stop=(kb == kmax - 1))
                                nc.tensor.matmul(
                                    o_ps, v_sb[:, kb, :], e_sb,
                                    start=(kb == 0), stop=(kb == kmax - 1))
                            r_sb = apool.tile([128, QG], F32, tag="r")
                            nc.vector.reciprocal(r_sb, z_ps)
                            nc.vector.tensor_mul(
                                ot_sb[h][:, qg * QG:(qg + 1) * QG],
                                o_ps, r_sb)

                with tc.tile_pool(name="wom", bufs=2) as wopool, \
                     tc.tile_pool(name="wos", bufs=6) as wosb, \
                     tc.tile_pool(name="wops", bufs=4, space="PSUM") as wps:
                    for cg in range(NCG):
                        wo_sb = wopool.tile([128, HPC, 512], FR, tag="wo")
                        for h in range(HPC):
                            nc.sync.dma_start(
                                out=wo_sb[:, h, :],
                                in_=wo[h * 128:(h + 1) * 128,
                                       cg * 512:(cg + 1) * 512])
                        for tb in range(NTB):
                            op = wps.tile([128, 512], F32, tag="op")
                            for h in range(HPC):
                                nc.tensor.matmul(
                                    op,
                                    ot_sb[h][:, tb * 128:(tb + 1) * 128],
                                    wo_sb[:, h, :],
                                    start=(h == 0), stop=(h == HPC - 1))
                            o_sb = wosb.tile([128, 512], F32, tag="osb")
                            nc.any.tensor_copy(o_sb, op)
                            nc.sync.dma_start(
                                out=out[t0 + tb * 128:t0 + (tb + 1) * 128,
                                        cg * 512:(cg + 1) * 512],
                                in_=o_sb)
    _split_waits(nc, mybir)
    return nc


def _get(variant):
    if variant not in _built:
        _built[variant] = (_build_causal() if variant == "causal"
                           else _build_generic())
    return _built[variant]


def _canonical_causal(mask_w, mask_b):
    tri = np.tril(np.ones((T, T), dtype=np.float32))
    if not np.array_equal(mask_w, tri):
        return False
    off = mask_b[tri == 0]
    if off.size and not (np.all(off <= -20000.0) and np.ptp(off) == 0):
        return False
    return bool(np.all(mask_b[tri == 1] == 0.0))


def _split8(a):
    import ml_dtypes
    F8 = ml_dtypes.float8_e4m3
    hi = a.astype(F8)
    lo = (a - hi.astype(np.float32)).astype(F8)
    return hi, lo


def _run(stm, wq, wk, wv, wo, cos, sin, mask_w, mask_b, trace=False):
    import ml_dtypes
    from concourse.bass_utils import run_bass_kernel_spmd

    x = np.ascontiguousarray(np.asarray(stm).reshape(BT, INNER))
    wq = np.asarray(wq); wk = np.asarray(wk); wv = np.asarray(wv)
    wo = np.asarray(wo)
    cos = np.asarray(cos); sin = np.asarray(sin)
    mask_w = np.asarray(mask_w); mask_b = np.asarray(mask_b)
    causal = _canonical_causal(mask_w, mask_b)

    if causal:
        xT = np.ascontiguousarray(x.T.astype(np.float32))
        xh, xl = _split8(xT)
        xpk = np.empty((INNER, B * NQG, 2 * QG), dtype=xh.dtype)
        xpk[:, :, :QG] = xh.reshape(INNER, B * NQG, QG)
        xpk[:, :, QG:] = xl.reshape(INNER, B * NQG, QG)
        xpk = np.ascontiguousarray(
            xpk.reshape(NIC, 128, B * NQG, 2 * QG).transpose(1, 2, 0, 3))
        cosT = np.ascontiguousarray((cos.T / SW).astype(np.float16))
        sinT = (sin.T / SW).astype(np.float32)
        sinT[: D // 2] *= -1.0
        sinTs = np.ascontiguousarray(sinT.astype(np.float16))
        ii, jj = np.meshgrid(np.arange(128), np.arange(128), indexing="ij")
        trineg = np.where(jj < ii, np.float32(-30000.0),
                          np.float32(0.0)).astype(np.float16)
        nc = _get("causal")
        in_maps = []
        for c in range(NCORES):
            wqkv = np.concatenate([
                wq[:, c * HPC * D:(c + 1) * HPC * D] * (ATTN_SCALE * SW),
                wk[:, c * D:(c + 1) * D] * SW,
                wv[:, c * D:(c + 1) * D] * SW], axis=1).astype(np.float32)
            whv, wlv = _split8(wqkv)
            whv = np.ascontiguousarray(
                whv.reshape(NIC, 128, -1).transpose(1, 0, 2))
            wlv = np.ascontiguousarray(
                wlv.reshape(NIC, 128, -1).transpose(1, 0, 2))
            wo8 = (wo[c * HPC * D:(c + 1) * HPC * D, :] * SO
                   ).astype(np.float32)
            wohv, wolv = _split8(wo8)
            wohv = np.ascontiguousarray(
                wohv.reshape(HPC, 128, -1).transpose(1, 0, 2))
            wolv = np.ascontiguousarray(
                wolv.reshape(HPC, 128, -1).transpose(1, 0, 2))
            in_maps.append({
                "xpk": xpk,
                "wh": np.ascontiguousarray(whv),
                "wl": np.ascontiguousarray(wlv),
                "woh": np.ascontiguousarray(wohv),
                "wol": np.ascontiguousarray(wolv),
                "cosT": cosT, "sinTs": sinTs, "trineg": trineg,
            })
        res = run_bass_kernel_spmd(nc, in_maps, core_ids=list(range(NCORES)),
                                   trace=trace)
        acc = res.results[0]["out"].astype(np.float64)
        for c in range(1, NCORES):
            acc += res.results[c]["out"]
        full = (acc / SO).astype(np.float32).reshape(B, T, H, D)
        return full, res

    # generic fallback (fp16)
    BF = np.float16
    xT = np.ascontiguousarray(x.T).astype(BF)
    wq_s = (wq * ATTN_SCALE).astype(BF)
    cosT = np.ascontiguousarray(cos.T)
    sinT = np.ascontiguousarray(sin.T)
    sinTs = sinT.copy()
    sinTs[: D // 2] *= -1.0
    nc = _get("generic")
    in_maps = []
    for c in range(NCORES):
        m = {
            "xT": xT,
            "wq": np.ascontiguousarray(wq_s[:, c * HPC * D:(c + 1) * HPC * D]),
            "wkv": np.ascontiguousarray(
                np.concatenate([wk[:, c * D:(c + 1) * D],
                                wv[:, c * D:(c + 1) * D]], axis=1)).astype(BF),
            "wo": np.ascontiguousarray(
                wo[c * HPC * D:(c + 1) * HPC * D, :]).astype(BF),
            "cosT": cosT,
            "sinTs": sinTs,
            "mwTf": np.ascontiguousarray(mask_w.T),
            "mbTf": np.ascontiguousarray(mask_b.T),
        }
        in_maps.append(m)
    res = run_bass_kernel_spmd(nc, in_maps, core_ids=list(range(NCORES)),
                               trace=trace)
    acc = res.results[0]["out"].astype(np.float64)
    for c in range(1, NCORES):
        acc += res.results[c]["out"]
    full = acc.astype(np.float32).reshape(B, T, H, D)
    return full, res


def kernel(stm, wq, wk, wv, wo, cos, sin, mask_w, mask_b):
    out, _ = _run(stm, wq, wk, wv, wo, cos, sin, mask_w, mask_b, trace=False)
    return out
